# revision 21
# baseline (speedup 1.0000x reference)
"""ChebConv (K=2) + temporal Conv1d GNN kernel for 8 Trainium2 NeuronCores.

Strategy (data-parallel over destination nodes):
  - Node axis padded to 50176 = 392 blocks of 128; blocks are assigned to
    (core, slot) pairs by a balance heuristic so all 8 cores share one
    static program with minimal padding.
  - Host precomputes w_hat (edge weights of -D^-1/2 A D^-1/2) and sorts the
    edge list by (dst block, src half, dst subblock-of-32); groups are
    padded to 16-edge granularity to the max count across cores.
  - Per block, the device gathers x rows of the edges' sources from a
    node-major copy of x via SWDGE dma_gather (fp8 rows padded to 512 B for
    full-rate DMA descriptors, or fp16 768 B rows), builds a sparse
    "one-hot * w_hat" matrix with broadcast-AP is_equal/mult, and reduces
    messages with TensorE matmuls in SWAPPED orientation
    (lhsT = gathered x, rhs = one-hot) so T1 lands feature-major in PSUM.
  - The Chebyshev combine + temporal conv collapse into dense per-node
    matmuls with host-prefolded fp16 weights, all in the transposed
    [feature, node] layout; LeakyReLU on-chip; output written transposed
    and unshuffled on the host.
"""

import numpy as np
import ml_dtypes

N = 50000
E = 1600000
W = 12
C = 32
WC = W * C            # 384
NCORES = 8
P = 128
NPAD = 50176          # 392 * 128
NB = NPAD // P        # 392
SLOTS = NB // NCORES  # 49
HALF = NPAD // 2      # 25088

G8 = True             # gather in fp8 (512B padded rows) vs fp16 (768B rows)
ELEM8 = 512

_cache = {}


def _plan_edges(src, dst):
    """Shared static plan + per-core edge data layouts."""
    blk = dst >> 7
    hh = (src >= HALF).astype(np.int64)
    sb = (dst >> 5) & 3
    gid = (blk * 2 + hh) * 4 + sb
    order = np.argsort(gid, kind="stable")
    src_s = src[order]
    dstl_s = (dst[order] & 127).astype(np.int64)
    counts = np.bincount(gid, minlength=NB * 8).reshape(NB, 2, 4)
    gstart = np.zeros(NB * 8 + 1, np.int64)
    np.cumsum(counts.reshape(-1), out=gstart[1:])

    # block -> (core, slot): snake by total count, then local-search to
    # minimize sum over (slot, h, s) of the max count across the 8 cores
    # (the shared-program padding cost).
    tot = counts.sum(axis=(1, 2))
    bo = np.argsort(-tot, kind="stable")
    slot_blocks = [list(bo[i * NCORES:(i + 1) * NCORES]) for i in range(SLOTS)]
    cflat = counts.reshape(NB, 8)

    def slot_cost(blks):
        return int(cflat[blks].max(axis=0).sum())

    costs = [slot_cost(b) for b in slot_blocks]
    rng = np.random.default_rng(12345)
    for _ in range(150000):
        i1, i2 = rng.integers(0, SLOTS, 2)
        if i1 == i2:
            continue
        k1, k2 = rng.integers(0, NCORES, 2)
        b1 = slot_blocks[i1]
        b2 = slot_blocks[i2]
        b1[k1], b2[k2] = b2[k2], b1[k1]
        c1, c2 = slot_cost(b1), slot_cost(b2)
        if c1 + c2 < costs[i1] + costs[i2]:
            costs[i1], costs[i2] = c1, c2
        else:
            b1[k1], b2[k2] = b2[k2], b1[k1]
    Bmap = np.array(slot_blocks, np.int64).T  # [NCORES, SLOTS]

    # per (slot, h, s): exact max count across cores; idx padding only at
    # the (slot, h) level up to a multiple of 16 (gather idx wrap).
    M = np.zeros((SLOTS, 2, 4), np.int64)
    for i in range(SLOTS):
        cnt = counts[Bmap[:, i]]          # [NCORES, 2, 4]
        M[i] = cnt.max(axis=0)
    NI = ((M.sum(axis=2) + 15) // 16) * 16  # [SLOTS, 2] num_idxs per h
    Jc = -(-NI // 128)                    # chunks per h

    # matmul stream per slot: list of (col, chunk_j, s) + start/stop per s
    streams = []
    JX = np.zeros(SLOTS, np.int64)
    for i in range(SLOTS):
        ents = []
        for h in range(2):
            jbase = 0 if h == 0 else int(Jc[i, 0])
            e0 = 0
            for s in range(4):
                e1 = e0 + int(M[i, h, s])
                jlo, jhi = e0 // 128, -(-e1 // 128)
                for j in range(jlo, jhi):
                    ents.append((jbase + j, s))
                e0 = e1
        first = {}
        last = {}
        for ci, (j, s) in enumerate(ents):
            first.setdefault(s, ci)
            last[s] = ci
        stream = [(ci, j, s, first[s] == ci, last[s] == ci)
                  for ci, (j, s) in enumerate(ents)]
        streams.append(stream)
        JX[i] = len(ents)

    iw = (NI[:, 0] + NI[:, 1]) // 16      # idx cols per slot
    iw_off = np.zeros(SLOTS + 1, np.int64)
    np.cumsum(iw, out=iw_off[1:])
    jx_off = np.zeros(SLOTS + 1, np.int64)
    np.cumsum(JX, out=jx_off[1:])

    return dict(order=order, src_s=src_s, dstl_s=dstl_s, gstart=gstart,
                Bmap=Bmap, M=M, NI=NI, Jc=Jc, streams=streams, JX=JX,
                iw=iw, iw_off=iw_off, jx_off=jx_off)


def _host_prep(x, A, Ew):
    src = np.asarray(A[0], np.int64)
    dst = np.asarray(A[1], np.int64)
    Ew = np.asarray(Ew, np.float32)

    deg = np.bincount(dst, weights=Ew.astype(np.float64), minlength=N).astype(np.float32)
    dinv = np.where(deg > 0, 1.0 / np.sqrt(np.maximum(deg, 1e-12)), 0.0).astype(np.float32)
    w_hat = (-dinv[src] * Ew * dinv[dst]).astype(np.float32)

    xrow = np.zeros((NPAD, WC), np.float32)
    xrow[:N] = np.asarray(x, np.float32).transpose(1, 0, 2).reshape(N, WC)
    xrow16 = xrow.astype(np.float16)
    xrow8 = None
    if G8:
        xrow8 = np.zeros((NPAD, ELEM8), ml_dtypes.float8_e4m3fn)
        xrow8[:, :WC] = xrow.astype(ml_dtypes.float8_e4m3fn)

    plan = _plan_edges(src, dst)
    what_s = w_hat[plan["order"]].astype(np.float16)
    src_s, dstl_s, gstart = plan["src_s"], plan["dstl_s"], plan["gstart"]
    M, NI, Jc, Bmap = plan["M"], plan["NI"], plan["Jc"], plan["Bmap"]
    streams, JX = plan["streams"], plan["JX"]
    iw_off, jx_off = plan["iw_off"], plan["jx_off"]

    IWT = int(iw_off[-1])
    JXT = int(jx_off[-1])
    idx16 = np.zeros((NCORES, 128, IWT), np.int16)
    dmwh = np.zeros((NCORES, 128, 2 * JXT), np.float16)
    xbs = np.zeros((NCORES, SLOTS, P, WC), np.float16)

    for c in range(NCORES):
        for i in range(SLOTS):
            b = int(Bmap[c, i])
            xbs[c, i] = xrow16[b * P:(b + 1) * P]
            JcT = int(Jc[i, 0] + Jc[i, 1])
            Dch = np.full(JcT * 128, 255, np.float16)   # dst&127 per chunk pos
            Wch = np.zeros(JcT * 128, np.float16)
            icol = int(iw_off[i])
            for h in range(2):
                L = int(NI[i, h])
                V = np.zeros(L, np.int16)
                cbase = 0 if h == 0 else int(Jc[i, 0]) * 128
                e0 = 0
                for s in range(4):
                    g = (b * 2 + h) * 4 + s
                    n = int(gstart[g + 1] - gstart[g])
                    sl = slice(int(gstart[g]), int(gstart[g] + n))
                    V[e0:e0 + n] = (src_s[sl] - h * HALF).astype(np.int16)
                    Dch[cbase + e0:cbase + e0 + n] = dstl_s[sl]
                    Wch[cbase + e0:cbase + e0 + n] = what_s[sl]
                    e0 += int(M[i, h, s])
                idx_blk = V.reshape(-1, 16).T               # [16, L/16]
                idx16[c, :, icol:icol + L // 16] = np.tile(idx_blk, (8, 1))
                icol += L // 16
            # dup-expanded dm/wh columns; dm pre-offset by -32*s so the
            # device compares every column against iota 0..31
            co = int(jx_off[i])
            jx = int(JX[i])
            for (ci, j, s, st, sp) in streams[i]:
                dmwh[c, :, 2 * co + ci] = Dch[j * 128:(j + 1) * 128] - 32 * s
                dmwh[c, :, 2 * co + jx + ci] = Wch[j * 128:(j + 1) * 128]

    return dict(xrow16=xrow16, xrow8=xrow8, idx16=idx16, dmwh=dmwh, xbs=xbs,
                plan=plan, IWT=IWT, JXT=JXT)


def _fold_weights(Wcheb, bcheb, Wconv, bconv):
    Wcheb = np.asarray(Wcheb, np.float32)
    bcheb = np.asarray(bcheb, np.float32)
    Wconv = np.asarray(Wconv, np.float32)
    bconv = np.asarray(bconv, np.float32)
    pairs = []
    for go in range(3):
        for gi in range(max(0, go - 1), min(3, go + 2)):
            for path in range(2):
                pairs.append((path, gi, go))
    mats = np.zeros((len(pairs), 128, 128), np.float32)
    for pi, (path, gi, go) in enumerate(pairs):
        for wo in range(4 * go, 4 * go + 4):
            for k in range(3):
                wi = wo + k - 1
                if not (4 * gi <= wi < 4 * gi + 4) or not (0 <= wi < W):
                    continue
                Cmat = Wcheb[wi, path] @ Wconv[:, :, k].T  # [ci, co]
                r0 = 32 * (wi - 4 * gi)
                c0 = 32 * (wo - 4 * go)
                mats[pi, r0:r0 + 32, c0:c0 + 32] = Cmat
    mats_sb = np.ascontiguousarray(
        mats.transpose(1, 0, 2).reshape(128, -1)).astype(np.float16)
    bias = np.zeros((12, 32), np.float32)
    for wo in range(12):
        bias[wo] = bconv.copy()
        for k in range(3):
            wi = wo + k - 1
            if 0 <= wi < W:
                bias[wo] += bcheb[wi] @ Wconv[:, :, k].T
    bias_sb = bias.reshape(3, 128).T.copy()  # [128, 3] fp32
    return mats_sb, bias_sb, pairs


def _build_program(plan, IWT, JXT, n_pairs):
    import concourse.bacc as bacc
    import concourse.tile as tile
    from concourse import mybir
    import concourse.bass as bass  # noqa

    M, NI, Jc = plan["M"], plan["NI"], plan["Jc"]
    streams, JX = plan["streams"], plan["JX"]
    iw_off, jx_off = plan["iw_off"], plan["jx_off"]
    JCmax = int((Jc[:, 0] + Jc[:, 1]).max())
    JXmax = int(JX.max())
    IWmax = int(((NI[:, 0] + NI[:, 1]) // 16).max())
    ELEM = ELEM8 if G8 else WC

    nc = bacc.Bacc("TRN2", target_bir_lowering=False, debug=False,
                   num_devices=NCORES)
    f16, f32, i16 = mybir.dt.float16, mybir.dt.float32, mybir.dt.int16
    f8 = mybir.dt.float8e4
    gdt = f8 if G8 else f16

    xrowg = nc.dram_tensor("xrowg", [NPAD, ELEM], gdt, kind="ExternalInput")
    xbs = nc.dram_tensor("xbs", [SLOTS, P, WC], f16, kind="ExternalInput")
    idx16 = nc.dram_tensor("idx16", [128, IWT], i16, kind="ExternalInput")
    dmwh = nc.dram_tensor("dmwh", [128, 2 * JXT], f16, kind="ExternalInput")
    mats = nc.dram_tensor("mats", [128, n_pairs * 128], f16, kind="ExternalInput")
    biasd = nc.dram_tensor("biasd", [128, 3], f32, kind="ExternalInput")
    iota4 = nc.dram_tensor("iota4", [128, 128], f16, kind="ExternalInput")
    ident = nc.dram_tensor("ident", [128, 128], f16, kind="ExternalInput")
    out_pc = nc.dram_tensor("out_pc", [128, 3, SLOTS * P], f16,
                            kind="ExternalOutput")

    pairs_by_go = [[], [], []]
    pi = 0
    for go in range(3):
        for gi in range(max(0, go - 1), min(3, go + 2)):
            for path in range(2):
                pairs_by_go[go].append((pi, gi, path))
                pi += 1

    with tile.TileContext(nc) as tc:
        with tc.tile_pool(name="const", bufs=1) as cp, \
             tc.tile_pool(name="sb", bufs=2) as sb, \
             tc.tile_pool(name="xgp", bufs=4) as xgp, \
             tc.tile_pool(name="osbp", bufs=2) as osbp, \
             tc.tile_pool(name="pst0", bufs=2, space="PSUM") as pst0, \
             tc.tile_pool(name="pst1", bufs=2, space="PSUM") as pst1, \
             tc.tile_pool(name="psy", bufs=2, space="PSUM") as psy:
            mats_t = cp.tile([128, n_pairs * 128], f16)
            nc.sync.dma_start(out=mats_t[:], in_=mats.ap())
            bias_t = cp.tile([128, 3], f32)
            nc.sync.dma_start(out=bias_t[:], in_=biasd.ap())
            iota_t = cp.tile([128, 128], f16)
            nc.sync.dma_start(out=iota_t[:], in_=iota4.ap())
            id_t = cp.tile([128, 128], f16)
            nc.sync.dma_start(out=id_t[:], in_=ident.ap())

            # preload all per-slot idx/dm/wh/xb data once, chunked at slot
            # boundaries so the first slots' gathers start early
            ib = (0, 3, 7, 12, 18, 25, 33, 41, SLOTS)
            idxall = cp.tile([128, IWT], i16)
            for a, bnd in zip(ib[:-1], ib[1:]):
                lo, hi = int(iw_off[a]), int(iw_off[bnd])
                nc.sync.dma_start(out=idxall[:, lo:hi],
                                  in_=idx16.ap()[:, lo:hi])
            db = (0, 8, 20, 34, SLOTS)
            dmwall = cp.tile([128, 2 * JXT], f16)
            for a, bnd in zip(db[:-1], db[1:]):
                lo, hi = 2 * int(jx_off[a]), 2 * int(jx_off[bnd])
                nc.sync.dma_start(out=dmwall[:, lo:hi],
                                  in_=dmwh.ap()[:, lo:hi])
            xball = cp.tile([128, SLOTS, WC], f16)
            nc.sync.dma_start(out=xball[:],
                              in_=xbs.ap().rearrange("i p f -> p i f"))

            osb = None
            for i in range(SLOTS):
                NI0, NI1 = int(NI[i, 0]), int(NI[i, 1])
                Jc0, Jc1 = int(Jc[i, 0]), int(Jc[i, 1])
                jx = int(JX[i])
                io = int(iw_off[i])
                co = int(jx_off[i])
                niw = (NI0 + NI1) // 16

                xg = xgp.tile([128, JCmax, ELEM], gdt, tag="xg")
                # zero partial tail chunks: the matmul reads all 128 edge
                # partitions; stale SBUF bits there could be Inf/NaN and
                # 0 * NaN = NaN even though wm masks those edges.
                if NI0 % 128:
                    nc.vector.memset(xg[:, Jc0 - 1, :WC], 0.0)
                if NI1 % 128:
                    nc.vector.memset(xg[:, Jc0 + Jc1 - 1, :WC], 0.0)
                # last two slots: split each gather at a chunk-aligned
                # midpoint so the reduce matmuls overlap the tail of the
                # final gather DMA (shrinks the pipeline drain).
                nsplit = 2 if i >= SLOTS - 2 else 1
                for hh, (jlo, Jch, NIh, iolo) in enumerate(
                        ((0, Jc0, NI0, io),
                         (Jc0, Jc1, NI1, io + NI0 // 16))):
                    if NIh == 0:
                        continue
                    base = (xrowg.ap()[0:HALF, :] if hh == 0
                            else xrowg.ap()[HALF:NPAD, :])
                    if nsplit == 2 and Jch >= 2:
                        mid = Jch // 2
                        nA = mid * 128
                        nc.gpsimd.dma_gather(
                            xg[:, jlo:jlo + mid, :], base,
                            idxall[:, iolo:iolo + nA // 16], nA, nA, ELEM,
                            single_packet=False)
                        nB = NIh - nA
                        nc.gpsimd.dma_gather(
                            xg[:, jlo + mid:jlo + Jch, :], base,
                            idxall[:, iolo + nA // 16:iolo + NIh // 16],
                            nB, nB, ELEM, single_packet=False)
                    else:
                        nc.gpsimd.dma_gather(
                            xg[:, jlo:jlo + Jch, :], base,
                            idxall[:, iolo:iolo + NIh // 16], NIh, NIh, ELEM,
                            single_packet=False)

                # one-hot * w_hat: dm columns are pre-offset by -32*s on the
                # host, so a single is_equal against iota 0..31 covers all
                # four dst sub-blocks; then one mult by w_hat.
                eq = sb.tile([128, JXmax, 32], f16, tag="eq")
                nc.vector.tensor_tensor(
                    out=eq[:, :jx, :],
                    in0=dmwall[:, 2 * co:2 * co + jx].unsqueeze(2)
                        .to_broadcast([128, jx, 32]),
                    in1=iota_t[:, 0:32].unsqueeze(1)
                        .to_broadcast([128, jx, 32]),
                    op=mybir.AluOpType.is_equal)
                wm = sb.tile([128, JXmax, 32], f16, tag="wm")
                nc.vector.tensor_tensor(
                    out=wm[:, :jx, :],
                    in0=eq[:, :jx, :],
                    in1=dmwall[:, 2 * co + jx:2 * co + 2 * jx].unsqueeze(2)
                        .to_broadcast([128, jx, 32]),
                    op=mybir.AluOpType.mult)

                # swapped reduce: T1^T [feat, dst] accumulated in PSUM
                # single accumulation group per PSUM tile: matmul start=True
                # zeroes the whole 2KB zero region, so only the very first
                # matmul starts and the very last stops.
                psum_t1 = pst1.tile([128, 3, 128], f32, space="PSUM", tag="t1")
                for fc in range(3):
                    for (ci, j, s, st, sp) in streams[i]:
                        nc.tensor.matmul(
                            out=psum_t1[:, fc, 32 * s:32 * s + 32],
                            lhsT=xg[:, j, 128 * fc:128 * fc + 128],
                            rhs=wm[:, ci, :],
                            start=(fc == 0 and ci == 0),
                            stop=(fc == 2 and ci == jx - 1))
                # T0^T via identity matmuls
                psum_t0 = pst0.tile([128, 3, 128], f32, space="PSUM", tag="t0")
                for fc in range(3):
                    nc.tensor.matmul(
                        out=psum_t0[:, fc, :],
                        lhsT=xball[:, i, 128 * fc:128 * fc + 128],
                        rhs=id_t[:],
                        start=(fc == 0), stop=(fc == 2))

                t1s = sb.tile([128, 3, 128], f16, tag="t1s")
                nc.scalar.copy(out=t1s[:], in_=psum_t1[:])
                t0s = sb.tile([128, 3, 128], f16, tag="t0s")
                nc.scalar.copy(out=t0s[:], in_=psum_t0[:])

                half = i % 2
                if half == 0:
                    osb = osbp.tile([128, 3, 256], f16, tag="osb")
                for go in range(3):
                    py = psy.tile([128, 128], f32, space="PSUM", tag="y")
                    plist = pairs_by_go[go]
                    for n_, (pi_, gi, path) in enumerate(plist):
                        rhs = (t0s if path == 0 else t1s)[:, gi, :]
                        nc.tensor.matmul(
                            out=py[:],
                            lhsT=mats_t[:, 128 * pi_:128 * pi_ + 128],
                            rhs=rhs,
                            start=(n_ == 0), stop=(n_ == len(plist) - 1))
                    ysl = osb[:, go, 128 * half:128 * half + 128]
                    nc.scalar.activation(
                        out=ysl, in_=py[:],
                        func=mybir.ActivationFunctionType.Identity,
                        bias=bias_t[:, go:go + 1], scale=1.0)
                oslice = osb[:, :, 128 * half:128 * half + 128]
                tl = sb.tile([128, 3, 128], f16, tag="tl")
                nc.vector.tensor_scalar_mul(out=tl[:], in0=oslice, scalar1=0.01)
                nc.vector.tensor_tensor(out=oslice, in0=oslice, in1=tl[:],
                                        op=mybir.AluOpType.max)
                if half == 1 or i == SLOTS - 1:
                    lo = (i - half) * P
                    nc.sync.dma_start(
                        out=out_pc.ap()[:, :, lo:lo + (half + 1) * P],
                        in_=osb[:, :, :(half + 1) * P])

    nc.compile()
    return nc


def kernel(x, A, Ew, Wcheb, bcheb, Wconv, bconv, batch_size=1):
    from concourse.bass_utils import run_bass_kernel_spmd

    prep = _host_prep(x, A, Ew)
    plan = prep["plan"]
    mats_sb, bias_sb, pairs = _fold_weights(Wcheb, bcheb, Wconv, bconv)

    key = (G8, prep["IWT"], prep["JXT"], tuple(plan["JX"].tolist()),
           tuple(plan["NI"].reshape(-1).tolist()))
    if key not in _cache:
        _cache[key] = _build_program(plan, prep["IWT"], prep["JXT"], len(pairs))
    nc = _cache[key]

    iota_np = np.tile(np.arange(128, dtype=np.float16)[None, :], (128, 1))
    ident_np = np.eye(128, dtype=np.float16)
    xg_src = prep["xrow8"] if G8 else prep["xrow16"]
    in_maps = []
    for c in range(NCORES):
        in_maps.append(dict(
            xrowg=xg_src, xbs=prep["xbs"][c], idx16=prep["idx16"][c],
            dmwh=prep["dmwh"][c], mats=mats_sb, biasd=bias_sb,
            iota4=iota_np, ident=ident_np))
    res = run_bass_kernel_spmd(nc, in_maps, core_ids=list(range(NCORES)))

    Bmap = plan["Bmap"]
    out = np.zeros((NPAD, W, C), np.float32)
    for c in range(NCORES):
        arr = np.asarray(res.results[c]["out_pc"], np.float32)  # [128,3,S*128]
        for i in range(SLOTS):
            b = int(Bmap[c, i])
            seg = arr[:, :, i * P:(i + 1) * P]          # [128(fo), 3(go), 128]
            blkout = seg.reshape(4, 32, 3, P).transpose(3, 2, 0, 1)
            out[b * P:(b + 1) * P] = blkout.reshape(P, W, C)
    return np.ascontiguousarray(out[:N])


# revision 22
# speedup vs baseline: 1.0012x; 1.0012x over previous
"""ChebConv (K=2) + temporal Conv1d GNN kernel for 8 Trainium2 NeuronCores.

Strategy (data-parallel over destination nodes):
  - Node axis padded to 50176 = 392 blocks of 128; blocks are assigned to
    (core, slot) pairs by a balance heuristic so all 8 cores share one
    static program with minimal padding.
  - Host precomputes w_hat (edge weights of -D^-1/2 A D^-1/2) and sorts the
    edge list by (dst block, src half, dst subblock-of-32); groups are
    padded to 16-edge granularity to the max count across cores.
  - Per block, the device gathers x rows of the edges' sources from a
    node-major copy of x via SWDGE dma_gather (fp8 rows padded to 512 B for
    full-rate DMA descriptors, or fp16 768 B rows), builds a sparse
    "one-hot * w_hat" matrix with broadcast-AP is_equal/mult, and reduces
    messages with TensorE matmuls in SWAPPED orientation
    (lhsT = gathered x, rhs = one-hot) so T1 lands feature-major in PSUM.
  - The Chebyshev combine + temporal conv collapse into dense per-node
    matmuls with host-prefolded fp16 weights, all in the transposed
    [feature, node] layout; LeakyReLU on-chip; output written transposed
    and unshuffled on the host.
"""

import numpy as np
import ml_dtypes

N = 50000
E = 1600000
W = 12
C = 32
WC = W * C            # 384
NCORES = 8
P = 128
NPAD = 50176          # 392 * 128
NB = NPAD // P        # 392
SLOTS = NB // NCORES  # 49
HALF = NPAD // 2      # 25088

G8 = True             # gather in fp8 (512B padded rows) vs fp16 (768B rows)
ELEM8 = 512

_cache = {}


def _plan_edges(src, dst):
    """Shared static plan + per-core edge data layouts."""
    blk = dst >> 7
    hh = (src >= HALF).astype(np.int64)
    sb = (dst >> 5) & 3
    gid = (blk * 2 + hh) * 4 + sb
    order = np.argsort(gid, kind="stable")
    src_s = src[order]
    dstl_s = (dst[order] & 127).astype(np.int64)
    counts = np.bincount(gid, minlength=NB * 8).reshape(NB, 2, 4)
    gstart = np.zeros(NB * 8 + 1, np.int64)
    np.cumsum(counts.reshape(-1), out=gstart[1:])

    # block -> (core, slot): snake by total count, then local-search to
    # minimize sum over (slot, h, s) of the max count across the 8 cores
    # (the shared-program padding cost).
    tot = counts.sum(axis=(1, 2))
    bo = np.argsort(-tot, kind="stable")
    slot_blocks = [list(bo[i * NCORES:(i + 1) * NCORES]) for i in range(SLOTS)]
    cflat = counts.reshape(NB, 8)

    def slot_cost(blks):
        return int(cflat[blks].max(axis=0).sum())

    costs = [slot_cost(b) for b in slot_blocks]
    rng = np.random.default_rng(12345)
    for _ in range(150000):
        i1, i2 = rng.integers(0, SLOTS, 2)
        if i1 == i2:
            continue
        k1, k2 = rng.integers(0, NCORES, 2)
        b1 = slot_blocks[i1]
        b2 = slot_blocks[i2]
        b1[k1], b2[k2] = b2[k2], b1[k1]
        c1, c2 = slot_cost(b1), slot_cost(b2)
        if c1 + c2 < costs[i1] + costs[i2]:
            costs[i1], costs[i2] = c1, c2
        else:
            b1[k1], b2[k2] = b2[k2], b1[k1]
    Bmap = np.array(slot_blocks, np.int64).T  # [NCORES, SLOTS]

    # per (slot, h, s): exact max count across cores; idx padding only at
    # the (slot, h) level up to a multiple of 16 (gather idx wrap).
    M = np.zeros((SLOTS, 2, 4), np.int64)
    for i in range(SLOTS):
        cnt = counts[Bmap[:, i]]          # [NCORES, 2, 4]
        M[i] = cnt.max(axis=0)
    NI = ((M.sum(axis=2) + 15) // 16) * 16  # [SLOTS, 2] num_idxs per h
    Jc = -(-NI // 128)                    # chunks per h

    # matmul stream per slot: list of (col, chunk_j, s) + start/stop per s
    streams = []
    JX = np.zeros(SLOTS, np.int64)
    for i in range(SLOTS):
        ents = []
        for h in range(2):
            jbase = 0 if h == 0 else int(Jc[i, 0])
            e0 = 0
            for s in range(4):
                e1 = e0 + int(M[i, h, s])
                jlo, jhi = e0 // 128, -(-e1 // 128)
                for j in range(jlo, jhi):
                    ents.append((jbase + j, s))
                e0 = e1
        first = {}
        last = {}
        for ci, (j, s) in enumerate(ents):
            first.setdefault(s, ci)
            last[s] = ci
        stream = [(ci, j, s, first[s] == ci, last[s] == ci)
                  for ci, (j, s) in enumerate(ents)]
        streams.append(stream)
        JX[i] = len(ents)

    iw = (NI[:, 0] + NI[:, 1]) // 16      # idx cols per slot
    iw_off = np.zeros(SLOTS + 1, np.int64)
    np.cumsum(iw, out=iw_off[1:])
    jx_off = np.zeros(SLOTS + 1, np.int64)
    np.cumsum(JX, out=jx_off[1:])

    return dict(order=order, src_s=src_s, dstl_s=dstl_s, gstart=gstart,
                Bmap=Bmap, M=M, NI=NI, Jc=Jc, streams=streams, JX=JX,
                iw=iw, iw_off=iw_off, jx_off=jx_off)


def _host_prep(x, A, Ew):
    src = np.asarray(A[0], np.int64)
    dst = np.asarray(A[1], np.int64)
    Ew = np.asarray(Ew, np.float32)

    deg = np.bincount(dst, weights=Ew.astype(np.float64), minlength=N).astype(np.float32)
    dinv = np.where(deg > 0, 1.0 / np.sqrt(np.maximum(deg, 1e-12)), 0.0).astype(np.float32)
    w_hat = (-dinv[src] * Ew * dinv[dst]).astype(np.float32)

    xrow = np.zeros((NPAD, WC), np.float32)
    xrow[:N] = np.asarray(x, np.float32).transpose(1, 0, 2).reshape(N, WC)
    xrow16 = xrow.astype(np.float16)
    xrow8 = None
    if G8:
        xrow8 = np.zeros((NPAD, ELEM8), ml_dtypes.float8_e4m3fn)
        xrow8[:, :WC] = xrow.astype(ml_dtypes.float8_e4m3fn)

    plan = _plan_edges(src, dst)
    what_s = w_hat[plan["order"]].astype(np.float16)
    src_s, dstl_s, gstart = plan["src_s"], plan["dstl_s"], plan["gstart"]
    M, NI, Jc, Bmap = plan["M"], plan["NI"], plan["Jc"], plan["Bmap"]
    streams, JX = plan["streams"], plan["JX"]
    iw_off, jx_off = plan["iw_off"], plan["jx_off"]

    IWT = int(iw_off[-1])
    JXT = int(jx_off[-1])
    idx16 = np.zeros((NCORES, 128, IWT), np.int16)
    dmwh = np.zeros((NCORES, 128, 2 * JXT), np.float16)
    xbs = np.zeros((NCORES, SLOTS, P, WC), np.float16)

    for c in range(NCORES):
        for i in range(SLOTS):
            b = int(Bmap[c, i])
            xbs[c, i] = xrow16[b * P:(b + 1) * P]
            JcT = int(Jc[i, 0] + Jc[i, 1])
            Dch = np.full(JcT * 128, 255, np.float16)   # dst&127 per chunk pos
            Wch = np.zeros(JcT * 128, np.float16)
            icol = int(iw_off[i])
            for h in range(2):
                L = int(NI[i, h])
                V = np.zeros(L, np.int16)
                cbase = 0 if h == 0 else int(Jc[i, 0]) * 128
                e0 = 0
                for s in range(4):
                    g = (b * 2 + h) * 4 + s
                    n = int(gstart[g + 1] - gstart[g])
                    sl = slice(int(gstart[g]), int(gstart[g] + n))
                    V[e0:e0 + n] = (src_s[sl] - h * HALF).astype(np.int16)
                    Dch[cbase + e0:cbase + e0 + n] = dstl_s[sl]
                    Wch[cbase + e0:cbase + e0 + n] = what_s[sl]
                    e0 += int(M[i, h, s])
                idx_blk = V.reshape(-1, 16).T               # [16, L/16]
                idx16[c, :, icol:icol + L // 16] = np.tile(idx_blk, (8, 1))
                icol += L // 16
            # dup-expanded dm/wh columns; dm pre-offset by -32*s so the
            # device compares every column against iota 0..31
            co = int(jx_off[i])
            jx = int(JX[i])
            for (ci, j, s, st, sp) in streams[i]:
                dmwh[c, :, 2 * co + ci] = Dch[j * 128:(j + 1) * 128] - 32 * s
                dmwh[c, :, 2 * co + jx + ci] = Wch[j * 128:(j + 1) * 128]

    return dict(xrow16=xrow16, xrow8=xrow8, idx16=idx16, dmwh=dmwh, xbs=xbs,
                plan=plan, IWT=IWT, JXT=JXT)


def _fold_weights(Wcheb, bcheb, Wconv, bconv):
    Wcheb = np.asarray(Wcheb, np.float32)
    bcheb = np.asarray(bcheb, np.float32)
    Wconv = np.asarray(Wconv, np.float32)
    bconv = np.asarray(bconv, np.float32)
    pairs = []
    for go in range(3):
        for gi in range(max(0, go - 1), min(3, go + 2)):
            for path in range(2):
                pairs.append((path, gi, go))
    mats = np.zeros((len(pairs), 128, 128), np.float32)
    for pi, (path, gi, go) in enumerate(pairs):
        for wo in range(4 * go, 4 * go + 4):
            for k in range(3):
                wi = wo + k - 1
                if not (4 * gi <= wi < 4 * gi + 4) or not (0 <= wi < W):
                    continue
                Cmat = Wcheb[wi, path] @ Wconv[:, :, k].T  # [ci, co]
                r0 = 32 * (wi - 4 * gi)
                c0 = 32 * (wo - 4 * go)
                mats[pi, r0:r0 + 32, c0:c0 + 32] = Cmat
    mats_sb = np.ascontiguousarray(
        mats.transpose(1, 0, 2).reshape(128, -1)).astype(np.float16)
    bias = np.zeros((12, 32), np.float32)
    for wo in range(12):
        bias[wo] = bconv.copy()
        for k in range(3):
            wi = wo + k - 1
            if 0 <= wi < W:
                bias[wo] += bcheb[wi] @ Wconv[:, :, k].T
    bias_sb = bias.reshape(3, 128).T.copy()  # [128, 3] fp32
    return mats_sb, bias_sb, pairs


def _build_program(plan, IWT, JXT, n_pairs):
    import concourse.bacc as bacc
    import concourse.tile as tile
    from concourse import mybir
    import concourse.bass as bass  # noqa

    M, NI, Jc = plan["M"], plan["NI"], plan["Jc"]
    streams, JX = plan["streams"], plan["JX"]
    iw_off, jx_off = plan["iw_off"], plan["jx_off"]
    JCmax = int((Jc[:, 0] + Jc[:, 1]).max())
    JXmax = int(JX.max())
    IWmax = int(((NI[:, 0] + NI[:, 1]) // 16).max())
    ELEM = ELEM8 if G8 else WC

    nc = bacc.Bacc("TRN2", target_bir_lowering=False, debug=False,
                   num_devices=NCORES)
    f16, f32, i16 = mybir.dt.float16, mybir.dt.float32, mybir.dt.int16
    f8 = mybir.dt.float8e4
    gdt = f8 if G8 else f16

    xrowg = nc.dram_tensor("xrowg", [NPAD, ELEM], gdt, kind="ExternalInput")
    xbs = nc.dram_tensor("xbs", [SLOTS, P, WC], f16, kind="ExternalInput")
    idx16 = nc.dram_tensor("idx16", [128, IWT], i16, kind="ExternalInput")
    dmwh = nc.dram_tensor("dmwh", [128, 2 * JXT], f16, kind="ExternalInput")
    mats = nc.dram_tensor("mats", [128, n_pairs * 128], f16, kind="ExternalInput")
    biasd = nc.dram_tensor("biasd", [128, 3], f32, kind="ExternalInput")
    iota4 = nc.dram_tensor("iota4", [128, 128], f16, kind="ExternalInput")
    ident = nc.dram_tensor("ident", [128, 128], f16, kind="ExternalInput")
    out_pc = nc.dram_tensor("out_pc", [128, 3, SLOTS * P], f16,
                            kind="ExternalOutput")

    pairs_by_go = [[], [], []]
    pi = 0
    for go in range(3):
        for gi in range(max(0, go - 1), min(3, go + 2)):
            for path in range(2):
                pairs_by_go[go].append((pi, gi, path))
                pi += 1

    with tile.TileContext(nc) as tc:
        with tc.tile_pool(name="const", bufs=1) as cp, \
             tc.tile_pool(name="sb", bufs=2) as sb, \
             tc.tile_pool(name="xgp", bufs=4) as xgp, \
             tc.tile_pool(name="osbp", bufs=2) as osbp, \
             tc.tile_pool(name="pst0", bufs=2, space="PSUM") as pst0, \
             tc.tile_pool(name="pst1", bufs=2, space="PSUM") as pst1, \
             tc.tile_pool(name="psy", bufs=2, space="PSUM") as psy:
            mats_t = cp.tile([128, n_pairs * 128], f16)
            nc.sync.dma_start(out=mats_t[:], in_=mats.ap())
            bias_t = cp.tile([128, 3], f32)
            nc.sync.dma_start(out=bias_t[:], in_=biasd.ap())
            iota_t = cp.tile([128, 128], f16)
            nc.sync.dma_start(out=iota_t[:], in_=iota4.ap())
            id_t = cp.tile([128, 128], f16)
            nc.sync.dma_start(out=id_t[:], in_=ident.ap())

            # preload all per-slot idx/dm/wh/xb data once, chunked at slot
            # boundaries so the first slots' gathers start early
            ib = (0, 3, 7, 12, 18, 25, 33, 41, SLOTS)
            idxall = cp.tile([128, IWT], i16)
            for a, bnd in zip(ib[:-1], ib[1:]):
                lo, hi = int(iw_off[a]), int(iw_off[bnd])
                nc.sync.dma_start(out=idxall[:, lo:hi],
                                  in_=idx16.ap()[:, lo:hi])
            db = (0, 8, 20, 34, SLOTS)
            dmwall = cp.tile([128, 2 * JXT], f16)
            for a, bnd in zip(db[:-1], db[1:]):
                lo, hi = 2 * int(jx_off[a]), 2 * int(jx_off[bnd])
                nc.sync.dma_start(out=dmwall[:, lo:hi],
                                  in_=dmwh.ap()[:, lo:hi])
            xball = cp.tile([128, SLOTS, WC], f16)
            nc.sync.dma_start(out=xball[:],
                              in_=xbs.ap().rearrange("i p f -> p i f"))

            osb = None
            for i in range(SLOTS):
                NI0, NI1 = int(NI[i, 0]), int(NI[i, 1])
                Jc0, Jc1 = int(Jc[i, 0]), int(Jc[i, 1])
                jx = int(JX[i])
                io = int(iw_off[i])
                co = int(jx_off[i])
                niw = (NI0 + NI1) // 16

                xg = xgp.tile([128, JCmax, ELEM], gdt, tag="xg")
                # zero partial tail chunks: the matmul reads all 128 edge
                # partitions; stale SBUF bits there could be Inf/NaN and
                # 0 * NaN = NaN even though wm masks those edges.
                if NI0 % 128:
                    nc.vector.memset(xg[:, Jc0 - 1, :WC], 0.0)
                if NI1 % 128:
                    nc.vector.memset(xg[:, Jc0 + Jc1 - 1, :WC], 0.0)
                if NI0:
                    nc.gpsimd.dma_gather(
                        xg[:, 0:Jc0, :], xrowg.ap()[0:HALF, :],
                        idxall[:, io:io + NI0 // 16], NI0, NI0, ELEM,
                        single_packet=False)
                if NI1:
                    nc.gpsimd.dma_gather(
                        xg[:, Jc0:Jc0 + Jc1, :], xrowg.ap()[HALF:NPAD, :],
                        idxall[:, io + NI0 // 16:io + niw], NI1, NI1, ELEM,
                        single_packet=False)

                # one-hot * w_hat: dm columns are pre-offset by -32*s on the
                # host, so a single is_equal against iota 0..31 covers all
                # four dst sub-blocks; then one mult by w_hat.
                eq = sb.tile([128, JXmax, 32], f16, tag="eq")
                nc.vector.tensor_tensor(
                    out=eq[:, :jx, :],
                    in0=dmwall[:, 2 * co:2 * co + jx].unsqueeze(2)
                        .to_broadcast([128, jx, 32]),
                    in1=iota_t[:, 0:32].unsqueeze(1)
                        .to_broadcast([128, jx, 32]),
                    op=mybir.AluOpType.is_equal)
                wm = sb.tile([128, JXmax, 32], f16, tag="wm")
                nc.vector.tensor_tensor(
                    out=wm[:, :jx, :],
                    in0=eq[:, :jx, :],
                    in1=dmwall[:, 2 * co + jx:2 * co + 2 * jx].unsqueeze(2)
                        .to_broadcast([128, jx, 32]),
                    op=mybir.AluOpType.mult)

                # swapped reduce: T1^T [feat, dst] accumulated in PSUM
                # single accumulation group per PSUM tile: matmul start=True
                # zeroes the whole 2KB zero region, so only the very first
                # matmul starts and the very last stops.
                psum_t1 = pst1.tile([128, 3, 128], f32, space="PSUM", tag="t1")
                for fc in range(3):
                    for (ci, j, s, st, sp) in streams[i]:
                        nc.tensor.matmul(
                            out=psum_t1[:, fc, 32 * s:32 * s + 32],
                            lhsT=xg[:, j, 128 * fc:128 * fc + 128],
                            rhs=wm[:, ci, :],
                            start=(fc == 0 and ci == 0),
                            stop=(fc == 2 and ci == jx - 1))
                # T0^T via identity matmuls
                psum_t0 = pst0.tile([128, 3, 128], f32, space="PSUM", tag="t0")
                for fc in range(3):
                    nc.tensor.matmul(
                        out=psum_t0[:, fc, :],
                        lhsT=xball[:, i, 128 * fc:128 * fc + 128],
                        rhs=id_t[:],
                        start=(fc == 0), stop=(fc == 2))

                t1s = sb.tile([128, 3, 128], f16, tag="t1s")
                nc.scalar.copy(out=t1s[:], in_=psum_t1[:])
                t0s = sb.tile([128, 3, 128], f16, tag="t0s")
                nc.scalar.copy(out=t0s[:], in_=psum_t0[:])

                half = i % 2
                if half == 0:
                    osb = osbp.tile([128, 3, 256], f16, tag="osb")
                for go in range(3):
                    py = psy.tile([128, 128], f32, space="PSUM", tag="y")
                    plist = pairs_by_go[go]
                    for n_, (pi_, gi, path) in enumerate(plist):
                        rhs = (t0s if path == 0 else t1s)[:, gi, :]
                        nc.tensor.matmul(
                            out=py[:],
                            lhsT=mats_t[:, 128 * pi_:128 * pi_ + 128],
                            rhs=rhs,
                            start=(n_ == 0), stop=(n_ == len(plist) - 1))
                    ysl = osb[:, go, 128 * half:128 * half + 128]
                    nc.scalar.activation(
                        out=ysl, in_=py[:],
                        func=mybir.ActivationFunctionType.Identity,
                        bias=bias_t[:, go:go + 1], scale=1.0)
                oslice = osb[:, :, 128 * half:128 * half + 128]
                tl = sb.tile([128, 3, 128], f16, tag="tl")
                nc.vector.tensor_scalar_mul(out=tl[:], in0=oslice, scalar1=0.01)
                nc.vector.tensor_tensor(out=oslice, in0=oslice, in1=tl[:],
                                        op=mybir.AluOpType.max)
                if half == 1 or i == SLOTS - 1:
                    lo = (i - half) * P
                    nc.sync.dma_start(
                        out=out_pc.ap()[:, :, lo:lo + (half + 1) * P],
                        in_=osb[:, :, :(half + 1) * P])

    nc.compile()
    return nc


def kernel(x, A, Ew, Wcheb, bcheb, Wconv, bconv, batch_size=1):
    from concourse.bass_utils import run_bass_kernel_spmd

    prep = _host_prep(x, A, Ew)
    plan = prep["plan"]
    mats_sb, bias_sb, pairs = _fold_weights(Wcheb, bcheb, Wconv, bconv)

    key = (G8, prep["IWT"], prep["JXT"], tuple(plan["JX"].tolist()),
           tuple(plan["NI"].reshape(-1).tolist()))
    if key not in _cache:
        _cache[key] = _build_program(plan, prep["IWT"], prep["JXT"], len(pairs))
    nc = _cache[key]

    iota_np = np.tile(np.arange(128, dtype=np.float16)[None, :], (128, 1))
    ident_np = np.eye(128, dtype=np.float16)
    xg_src = prep["xrow8"] if G8 else prep["xrow16"]
    in_maps = []
    for c in range(NCORES):
        in_maps.append(dict(
            xrowg=xg_src, xbs=prep["xbs"][c], idx16=prep["idx16"][c],
            dmwh=prep["dmwh"][c], mats=mats_sb, biasd=bias_sb,
            iota4=iota_np, ident=ident_np))
    res = run_bass_kernel_spmd(nc, in_maps, core_ids=list(range(NCORES)))

    Bmap = plan["Bmap"]
    out = np.zeros((NPAD, W, C), np.float32)
    for c in range(NCORES):
        arr = np.asarray(res.results[c]["out_pc"], np.float32)  # [128,3,S*128]
        for i in range(SLOTS):
            b = int(Bmap[c, i])
            seg = arr[:, :, i * P:(i + 1) * P]          # [128(fo), 3(go), 128]
            blkout = seg.reshape(4, 32, 3, P).transpose(3, 2, 0, 1)
            out[b * P:(b + 1) * P] = blkout.reshape(P, W, C)
    return np.ascontiguousarray(out[:N])
